# revision 30
# baseline (speedup 1.0000x reference)
"""Trainium2 Bass kernel for the paired-view ("flip") multi-head attention module.

Full computation (reference semantics, B=2 P=2 S=1024 D=1024 H=16):
    q/k/v = Linear(x) -> [B,P,H,S,DK]
    left  = softmax(q k^T / 8 + mask) v          (same pair index)
    right = softmax(q k_flip^T / 8 + mask) v_flip (pair index swapped)
    out   = (left + 0.1*tanh(right)) @ Wo.T + bo

Sharding over 8 NeuronCores: data-parallel on B (2 groups of 4 cores),
tensor-parallel on heads within a group (4 heads/core, 256 channels).
Each core computes its heads' projections (column-parallel), full attention
for its heads over both pair views, and a row-parallel partial of the output
projection.  The host sums the 4 partials per batch and adds bo.

Key layout trick: scores are computed TRANSPOSED ([k, q] instead of [q, k])
so softmax's exp is orientation-free and the attention-value product needs
no on-chip transposes; row sums come free via an extra ones-column in V.
Matmuls run in bf16 (fp32 PSUM accumulate); attention is ScalarE-exp-bound,
so projections and the p0 output projection are interleaved into the
ACT-saturated stream, and softmax reciprocals are batched by folding the
sums rows onto 32 partitions.
"""

import numpy as np

import concourse.bass as bass
import concourse.tile as tile
from concourse import bacc, mybir
from concourse.bass_utils import run_bass_kernel_spmd

F32 = mybir.dt.float32
F32R = mybir.dt.float32r
BF16 = mybir.dt.bfloat16
I32 = mybir.dt.int32

# per-stage matmul input dtypes (both operands of a matmul must match)
X_DT = BF16    # projection inputs: xT staging + Wq/Wk/Wv
QK_DT = BF16   # q/k tiles feeding the scores matmul
AV_DT = BF16   # exp(scores) + v_aug feeding the AV matmul
OUT_DT = BF16  # combine + Wo feeding the output projection
AF = mybir.ActivationFunctionType
OP = mybir.AluOpType

B, P, S, D, H = 2, 2, 1024, 1024, 16
DK = D // H          # 64
NCORES = 8
GROUP = 4            # cores per batch entry
NH = H // GROUP      # 4 local heads per core
CH = NH * DK         # 256 local channels
R = P * S            # 2048 rows per batch entry
KC = 8               # d_model chunks of 128
RB = 4               # row banks of 512
MASK_NEG = 60.0      # exp(-60) == 0 relative to any sum


def _emit(nc, tc, xq, xk, xv, wq, wk, wv, wo, bq, bk, bv, mask, out_d):
    from contextlib import ExitStack

    with ExitStack() as ctx:
        sb = ctx.enter_context(tc.tile_pool(name="sb", bufs=1))
        ps = ctx.enter_context(tc.tile_pool(name="ps", bufs=1, space="PSUM"))
        _body(nc, sb, ps, xq, xk, xv, wq, wk, wv, wo, bq, bk, bv, mask, out_d)


def _body(nc, sb, ps, xq, xk, xv, wq, wk, wv, wo, bq, bk, bv, mask, out_d):
    # ---- constants ----------------------------------------------------
    wq_sb = sb.tile([128, KC * CH], X_DT, name="wq_sb")
    wk_sb = sb.tile([128, KC * CH], X_DT, name="wk_sb")
    wv_sb = sb.tile([128, KC * CH], X_DT, name="wv_sb")
    for t_d, t_s in ((wq, wq_sb), (wk, wk_sb), (wv, wv_sb)):
        nc.gpsimd.dma_start(
            out=t_s[:].rearrange("p (kc c) -> p kc c", kc=KC),
            in_=t_d[:].rearrange("(kc p) c -> p kc c", p=128),
        )
    wo_sb = sb.tile([128, 2 * D], OUT_DT, name="wo_sb")
    nc.gpsimd.dma_start(
        out=wo_sb[:].rearrange("p (kk c) -> p kk c", kk=2),
        in_=wo[:].rearrange("(kk p) c -> p kk c", p=128),
    )

    bq_sb = sb.tile([128, 2], F32, name="bq_sb")
    bk_sb = sb.tile([128, 2], F32, name="bk_sb")
    nc.sync.dma_start(out=bq_sb[:], in_=bq[:].rearrange("(mo p) -> p mo", p=128))
    nc.sync.dma_start(out=bk_sb[:], in_=bk[:].rearrange("(mo p) -> p mo", p=128))
    bv_row = sb.tile([1, CH], F32, name="bv_row")
    nc.sync.dma_start(out=bv_row[:], in_=bv[None, :])
    bv_bc = sb.tile([128, CH], F32, name="bv_bc")
    nc.gpsimd.partition_broadcast(bv_bc[:], bv_row[:])

    # mask as a per-row 0/1 multiplier on v_aug (kills masked keys in both
    # the attention numerator and the ones-column denominator)
    mask_sb = sb.tile([128, 2 * KC], I32, name="mask_sb")
    nc.sync.dma_start(
        out=mask_sb[:],
        in_=mask[:].rearrange("pp (kc p) -> p pp kc", p=128),
    )
    mbias = sb.tile([128, 2 * KC], F32, name="mbias")
    # (mask - 1) * MASK_NEG: 0 where mask==1, -MASK_NEG where mask==0
    nc.vector.tensor_scalar(
        out=mbias[:], in0=mask_sb[:], scalar1=-1, scalar2=MASK_NEG,
        op0=OP.add, op1=OP.mult,
    )

    ones_t = sb.tile([1, 64], F32, name="ones_t")
    nc.vector.memset(ones_t[:], 1.0)

    # ---- projections --------------------------------------------------
    # qT/kT: [o_local, p*S + s] in 2 tiles of 128 channels (2 heads each)
    qT = [sb.tile([128, R], QK_DT, name=f"qT{mo}") for mo in range(2)]
    kT = [sb.tile([128, R], QK_DT, name=f"kT{mo}") for mo in range(2)]
    # v_aug: [r_local, rc(16) x (h(4) x 65)]; col h*65+64 holds ones
    v_aug = sb.tile([128, 16 * NH * 65], AV_DT, name="v_aug")
    nc.gpsimd.memset(v_aug[:], 1.0)

    _stage_cache = {}

    def proj_chunk(kind, rb, part=None, lead=False):
        src_d = {"q": xq, "k": xk, "v": xv}[kind]
        w_sb = {"q": wq_sb, "k": wk_sb, "v": wv_sb}[kind]
        mos = (0, 1) if part is None else (part,)
        rss = (0, 1, 2, 3) if part is None else ((0, 1) if part == 0 else (2, 3))
        if (kind, rb) in _stage_cache:
            stage = _stage_cache[(kind, rb)]
        else:
            stage = sb.tile([128, KC * 512], X_DT, name="stage", tag="stage", bufs=6)
            _stage_cache[(kind, rb)] = stage
            engs = (nc.sync, nc.scalar) if lead else (nc.sync, nc.gpsimd)
            for half in range(2):
                eng = engs[half]
                eng.dma_start(
                    out=stage[:, half * 2048 : (half + 1) * 2048].rearrange(
                        "p (kc c) -> p kc c", kc=KC // 2
                    ),
                    in_=src_d[
                        half * 512 : 1024 if half else 512,
                        rb * 512 : (rb + 1) * 512,
                    ].rearrange("(kc p) c -> p kc c", p=128),
                )
        if kind in ("q", "k"):
            dst, b_sb = (qT, bq_sb) if kind == "q" else (kT, bk_sb)
            for mo in mos:
                pp_t = ps.tile([128, 512], F32, name="ps_proj", tag="ps_proj", bufs=2)
                for kc in range(KC):
                    nc.tensor.matmul(
                        pp_t[:],
                        w_sb[:, kc * CH + mo * 128 : kc * CH + (mo + 1) * 128],
                        stage[:, kc * 512 : (kc + 1) * 512],
                        start=(kc == 0),
                        stop=(kc == KC - 1),
                    )
                nc.vector.tensor_scalar(
                    out=dst[mo][:, rb * 512 : (rb + 1) * 512],
                    in0=pp_t[:],
                    scalar1=b_sb[:, mo : mo + 1],
                    scalar2=None,
                    op0=OP.add,
                )
        else:
            for rs in rss:
                rc = rb * 4 + rs
                pv_t = ps.tile([128, CH], F32, name="ps_v", tag="ps_proj", bufs=2)
                for kc in range(KC):
                    nc.tensor.matmul(
                        pv_t[:],
                        stage[:, kc * 512 + rs * 128 : kc * 512 + (rs + 1) * 128],
                        wv_sb[:, kc * CH : (kc + 1) * CH],
                        start=(kc == 0),
                        stop=(kc == KC - 1),
                    )
                dst_ap = v_aug[
                    :, rc * NH * 65 : (rc + 1) * NH * 65
                ].rearrange("p (h x) -> p h x", h=NH)[:, :, 0:DK]
                nc.vector.tensor_tensor(
                    out=dst_ap,
                    in0=pv_t[:].rearrange("p (h x) -> p h x", h=NH),
                    in1=bv_bc[:].rearrange("p (h x) -> p h x", h=NH),
                    op=OP.add,
                )

    # ---- attention building blocks ------------------------------------
    comb = [sb.tile([128, R], OUT_DT, name=f"comb{kk}") for kk in range(2)]

    def qk_part(p, h, side):
        pp = p if side == 0 else 1 - p
        mo, po = h // 2, (h % 2) * 64
        ex = [
            sb.tile([128, 4096], AV_DT, name="ex", tag="ex", bufs=3)
            for _ in range(2)
        ]
        for kc in range(KC):
            ss_t = ps.tile([128, 1024], F32, name="ps_s", tag="ps_s", bufs=3)
            for qb in range(2):
                nc.tensor.matmul(
                    ss_t[:, qb * 512 : (qb + 1) * 512],
                    kT[mo][po : po + 64, pp * S + kc * 128 : pp * S + (kc + 1) * 128],
                    qT[mo][po : po + 64, p * S + qb * 512 : p * S + (qb + 1) * 512],
                    start=True,
                    stop=True,
                )
            nc.scalar.activation(
                ex[kc // 4][:, (kc % 4) * 1024 : (kc % 4 + 1) * 1024],
                ss_t[:],
                AF.Exp,
                bias=mbias[:, pp * KC + kc : pp * KC + kc + 1],
                scale=0.125,
            )
        return ex

    def av_part(p, h, side, ex):
        pp = p if side == 0 else 1 - p
        av = sb.tile([65, S], F32, name="av", tag="avT", bufs=5)
        for qb in range(2):
            pa_t = ps.tile([65, 512], F32, name="ps_av", tag="ps_proj", bufs=2)
            for kc in range(KC):
                nc.tensor.matmul(
                    pa_t[:],
                    v_aug[:, (pp * KC + kc) * NH * 65 + h * 65 : (pp * KC + kc) * NH * 65 + (h + 1) * 65],
                    ex[kc // 4][:, (kc % 4) * 1024 + qb * 512 : (kc % 4) * 1024 + (qb + 1) * 512],
                    start=(kc == 0),
                    stop=(kc == KC - 1),
                )
            nc.vector.tensor_copy(av[:, qb * 512 : (qb + 1) * 512], pa_t[:])
        return av

    def combo(p, h, side):
        return av_part(p, h, side, qk_part(p, h, side))

    _pair = {}

    def combine(p, h, avL, avR, pe_bc=False):
        srs = sb.tile([32, 64], F32, name="srs", tag="srs", bufs=2)
        nc.sync.dma_start(
            out=srs[0:16, :], in_=avL[64:65, :].rearrange("p (m e) -> p m e", e=64)
        )
        nc.sync.dma_start(
            out=srs[16:32, :], in_=avR[64:65, :].rearrange("p (m e) -> p m e", e=64)
        )
        rrs = sb.tile([32, 64], F32, name="rrs", tag="rrs", bufs=2)
        nc.vector.reciprocal(rrs[:], srs[:])
        rr2 = sb.tile([1, 2 * S], F32, name="rr2", tag="rrow", bufs=2)
        nc.sync.dma_start(
            out=rr2[:, 0:S].rearrange("p (m e) -> p m e", e=64), in_=rrs[0:16, :]
        )
        nc.sync.dma_start(
            out=rr2[:, S : 2 * S].rearrange("p (m e) -> p m e", e=64), in_=rrs[16:32, :]
        )
        def part2():
            po = (h % 2) * 64
            if pe_bc:
                bcA = ps.tile([64, S], F32, name="bcA", tag="ps_s", bufs=3)
                bcB = ps.tile([64, S], F32, name="bcB", tag="ps_s", bufs=3)
                for c in range(4):
                    dst = bcA if c < 2 else bcB
                    nc.tensor.matmul(
                        dst[:, (c % 2) * 512 : (c % 2 + 1) * 512],
                        ones_t[:].bitcast(F32R),
                        rr2[0:1, c * 512 : (c + 1) * 512].bitcast(F32R),
                        start=True,
                        stop=True,
                    )
                bcL_ap, bcR_ap = bcA[:], bcB[:]
            else:
                bc2 = sb.tile([64, 2 * S], F32, name="bc2", tag="bc", bufs=2)
                nc.gpsimd.partition_broadcast(bc2[:], rr2[:])
                bcL_ap, bcR_ap = bc2[:, 0:S], bc2[:, S : 2 * S]
            if h % 2 == 0:
                t1p = sb.tile([128, S], F32, name="t1p", tag="t1", bufs=2)
                t2p = sb.tile([128, S], F32, name="t2p", tag="t2", bufs=2)
                _pair[(p, h // 2)] = (t1p, t2p)
            else:
                t1p, t2p = _pair[(p, h // 2)]
            nc.vector.tensor_tensor(
                out=t1p[po : po + 64, :], in0=avL[0:64, :], in1=bcL_ap, op=OP.mult
            )
            nc.vector.tensor_tensor(
                out=t2p[po : po + 64, :], in0=avR[0:64, :], in1=bcR_ap, op=OP.mult
            )
            if h % 2 == 1:
                t3p = sb.tile([128, S], F32, name="t3p", tag="t3", bufs=2)
                nc.scalar.activation(t3p[:], t2p[:], AF.Tanh)
                nc.vector.scalar_tensor_tensor(
                    out=comb[h // 2][:, p * S : (p + 1) * S],
                    in0=t3p[:],
                    scalar=0.1,
                    in1=t1p[:],
                    op0=OP.mult,
                    op1=OP.add,
                )

        return part2

    def outproj_rc(p, rc):
        od = sb.tile([128, D], F32, name="od", tag="od", bufs=2)
        for ob in range(2):
            po_t = ps.tile([128, 512], F32, name="ps_o", tag="ps_proj", bufs=2)
            for kk in range(2):
                nc.tensor.matmul(
                    po_t[:],
                    comb[kk][:, p * S + rc * 128 : p * S + (rc + 1) * 128],
                    wo_sb[:, kk * D + ob * 512 : kk * D + (ob + 1) * 512],
                    start=(kk == 0),
                    stop=(kk == 1),
                )
            nc.vector.tensor_copy(od[:, ob * 512 : (ob + 1) * 512], po_t[:])
        (nc.sync if rc % 2 == 0 else nc.gpsimd).dma_start(
            out=out_d[p * S + rc * 128 : p * S + (rc + 1) * 128, :], in_=od[:]
        )

    # ---- schedule -----------------------------------------------------
    # lead-in: only the mo=0 halves of k/q (all heads 0-1 need) so the
    # first combo starts ~14us earlier; v hides inside the first combo
    for kind, rb in (("k", 0), ("k", 1), ("q", 0), ("q", 1)):
        proj_chunk(kind, rb, part=0, lead=True)
    ex00 = qk_part(0, 0, 0)
    proj_chunk("v", 0)
    proj_chunk("v", 1)
    av0 = {0: av_part(0, 0, 0, ex00)}
    proj_chunk("k", 0, part=1)
    proj_chunk("k", 1, part=1)
    av0[1] = combo(0, 1, 0)
    proj_chunk("q", 0, part=1)
    proj_chunk("q", 1, part=1)
    av0[2] = combo(0, 2, 0)
    proj_chunk("k", 2, part=0)
    proj_chunk("k", 2, part=1)
    av0[3] = combo(0, 3, 0)
    proj_chunk("k", 3, part=0)
    proj_chunk("k", 3, part=1)

    # p0 side-1: v's p1 half must land before the first AV that reads it
    ex001 = qk_part(0, 0, 1)
    proj_chunk("v", 2, part=0)
    proj_chunk("v", 2, part=1)
    proj_chunk("v", 3, part=0)
    proj_chunk("v", 3, part=1)
    avR = av_part(0, 0, 1, ex001)
    pending = combine(0, 0, av0[0], avR)
    q_rest = [("q", 2, 0), ("q", 2, 1), ("q", 3, 0), ("q", 3, 1)]
    for h in range(1, NH):
        avR = combo(0, h, 1)
        if pending:
            pending()
        for _ in range(2):
            if q_rest:
                proj_chunk(*q_rest.pop(0))
        pending = combine(0, h, av0[h], avR)

    # p1 attention: p0's output projection fills the ACT-bound stream;
    # combine part-2 chains are deferred one combo so QK work covers them
    for h in range(NH):
        avL = combo(1, h, 0)
        if pending:
            pending()
        if h < 3:
            outproj_rc(0, 2 * h)
        avR = combo(1, h, 1)
        if h < 3:
            outproj_rc(0, 2 * h + 1)
        pending = combine(1, h, avL, avR, pe_bc=(h == 3))
    for rc in (6, 7):
        outproj_rc(0, rc)
    pending()
    for rc in range(8):
        outproj_rc(1, rc)


_CACHED = None


def _build():
    global _CACHED
    if _CACHED is not None:
        return _CACHED
    nc = bacc.Bacc("TRN2", target_bir_lowering=False, debug=False)
    xq = nc.dram_tensor("xq", [D, R], X_DT, kind="ExternalInput")
    xk = nc.dram_tensor("xk", [D, R], X_DT, kind="ExternalInput")
    xv = nc.dram_tensor("xv", [D, R], X_DT, kind="ExternalInput")
    wq = nc.dram_tensor("wq", [D, CH], X_DT, kind="ExternalInput")
    wk = nc.dram_tensor("wk", [D, CH], X_DT, kind="ExternalInput")
    wv = nc.dram_tensor("wv", [D, CH], X_DT, kind="ExternalInput")
    wo = nc.dram_tensor("wo", [CH, D], OUT_DT, kind="ExternalInput")
    bq = nc.dram_tensor("bq", [CH], F32, kind="ExternalInput")
    bk = nc.dram_tensor("bk", [CH], F32, kind="ExternalInput")
    bv = nc.dram_tensor("bv", [CH], F32, kind="ExternalInput")
    mask = nc.dram_tensor("mask", [P, S], I32, kind="ExternalInput")
    out_d = nc.dram_tensor("out", [R, D], F32, kind="ExternalOutput")
    with tile.TileContext(nc) as tc:
        _emit(nc, tc, xq, xk, xv, wq, wk, wv, wo, bq, bk, bv, mask, out_d)
    nc.compile()
    _CACHED = nc
    return nc


def _in_maps(query, key, value, mask, Wq, bq, Wk, bk, Wv, bv, Wo):
    xnp = mybir.dt.np(X_DT)
    onp = mybir.dt.np(OUT_DT)
    f32 = lambda a: np.ascontiguousarray(np.asarray(a, dtype=np.float32))
    xdt = lambda a: np.ascontiguousarray(np.asarray(a).astype(xnp))
    odt = lambda a: np.ascontiguousarray(np.asarray(a).astype(onp))
    query, key, value = f32(query), f32(key), f32(value)
    Wq, Wk, Wv, Wo = f32(Wq), f32(Wk), f32(Wv), f32(Wo)
    bq, bk, bv = f32(bq), f32(bk), f32(bv)
    mask = np.ascontiguousarray(np.asarray(mask, dtype=np.int32))

    xqT = [xdt(query[b].reshape(R, D).T) for b in range(B)]
    xkT = [xdt(key[b].reshape(R, D).T) for b in range(B)]
    xvT = [xdt(value[b].reshape(R, D).T) for b in range(B)]

    maps = []
    for c in range(NCORES):
        b, hg = divmod(c, GROUP)
        ch = slice(hg * CH, (hg + 1) * CH)
        maps.append(
            {
                "xq": xqT[b],
                "xk": xkT[b],
                "xv": xvT[b],
                "wq": xdt(Wq[ch, :].T),
                "wk": xdt(Wk[ch, :].T),
                "wv": xdt(Wv[ch, :].T),
                "wo": odt(Wo[:, ch].T),
                "bq": bq[ch],
                "bk": bk[ch],
                "bv": bv[ch],
                "mask": mask[b, :, 0, :],
            }
        )
    return maps


def _run(in_maps, **kwargs):
    nc = _build()
    return run_bass_kernel_spmd(nc, in_maps, core_ids=list(range(NCORES)), **kwargs)


def kernel(query, key, value, mask, Wq, bq, Wk, bk, Wv, bv, Wo, bo):
    res = _run(_in_maps(query, key, value, mask, Wq, bq, Wk, bk, Wv, bv, Wo))
    bo = np.asarray(bo, dtype=np.float32)
    out = np.zeros((B, P, S, D), dtype=np.float32)
    for c in range(NCORES):
        b = c // GROUP
        out[b] += res.results[c]["out"].reshape(P, S, D)
    out += bo
    return out


# revision 31
# speedup vs baseline: 1.0611x; 1.0611x over previous
"""Trainium2 Bass kernel for the paired-view ("flip") multi-head attention module.

Full computation (reference semantics, B=2 P=2 S=1024 D=1024 H=16):
    q/k/v = Linear(x) -> [B,P,H,S,DK]
    left  = softmax(q k^T / 8 + mask) v          (same pair index)
    right = softmax(q k_flip^T / 8 + mask) v_flip (pair index swapped)
    out   = (left + 0.1*tanh(right)) @ Wo.T + bo

Sharding over 8 NeuronCores: data-parallel on B (2 groups of 4 cores),
tensor-parallel on heads within a group (4 heads/core, 256 channels).
Each core computes its heads' projections (column-parallel), full attention
for its heads over both pair views, and a row-parallel partial of the output
projection.  The host sums the 4 partials per batch and adds bo.

Key layout trick: scores are computed TRANSPOSED ([k, q] instead of [q, k])
so softmax's exp is orientation-free and the attention-value product needs
no on-chip transposes; row sums come free via an extra ones-column in V.
Matmuls run in bf16 (fp32 PSUM accumulate); attention is ScalarE-exp-bound,
so projections and the p0 output projection are interleaved into the
ACT-saturated stream, and softmax reciprocals are batched by folding the
sums rows onto 32 partitions.
"""

import numpy as np

import concourse.bass as bass
import concourse.tile as tile
from concourse import bacc, mybir
from concourse.bass_utils import run_bass_kernel_spmd

F32 = mybir.dt.float32
F32R = mybir.dt.float32r
BF16 = mybir.dt.bfloat16
I32 = mybir.dt.int32

# per-stage matmul input dtypes (both operands of a matmul must match)
X_DT = BF16    # projection inputs: xT staging + Wq/Wk/Wv
QK_DT = BF16   # q/k tiles feeding the scores matmul
AV_DT = BF16   # exp(scores) + v_aug feeding the AV matmul
OUT_DT = BF16  # combine + Wo feeding the output projection
AF = mybir.ActivationFunctionType
OP = mybir.AluOpType

B, P, S, D, H = 2, 2, 1024, 1024, 16
DK = D // H          # 64
NCORES = 8
GROUP = 4            # cores per batch entry
NH = H // GROUP      # 4 local heads per core
CH = NH * DK         # 256 local channels
R = P * S            # 2048 rows per batch entry
KC = 8               # d_model chunks of 128
RB = 4               # row banks of 512
MASK_NEG = 60.0      # exp(-60) == 0 relative to any sum


def _emit(nc, tc, xq, xk, xv, wq, wk, wv, wo, bq, bk, bv, mask, out_d):
    from contextlib import ExitStack

    with ExitStack() as ctx:
        sb = ctx.enter_context(tc.tile_pool(name="sb", bufs=1))
        ps = ctx.enter_context(tc.tile_pool(name="ps", bufs=1, space="PSUM"))
        _body(nc, sb, ps, xq, xk, xv, wq, wk, wv, wo, bq, bk, bv, mask, out_d)


def _body(nc, sb, ps, xq, xk, xv, wq, wk, wv, wo, bq, bk, bv, mask, out_d):
    # ---- constants ----------------------------------------------------
    wq_sb = sb.tile([128, KC * CH], X_DT, name="wq_sb")
    wk_sb = sb.tile([128, KC * CH], X_DT, name="wk_sb")
    wv_sb = sb.tile([128, KC * CH], X_DT, name="wv_sb")
    for t_d, t_s in ((wq, wq_sb), (wk, wk_sb), (wv, wv_sb)):
        nc.gpsimd.dma_start(
            out=t_s[:].rearrange("p (kc c) -> p kc c", kc=KC),
            in_=t_d[:].rearrange("(kc p) c -> p kc c", p=128),
        )
    wo_sb = sb.tile([128, 2 * D], OUT_DT, name="wo_sb")
    nc.gpsimd.dma_start(
        out=wo_sb[:].rearrange("p (kk c) -> p kk c", kk=2),
        in_=wo[:].rearrange("(kk p) c -> p kk c", p=128),
    )

    bq_sb = sb.tile([128, 2], F32, name="bq_sb")
    bk_sb = sb.tile([128, 2], F32, name="bk_sb")
    nc.sync.dma_start(out=bq_sb[:], in_=bq[:].rearrange("(mo p) -> p mo", p=128))
    nc.sync.dma_start(out=bk_sb[:], in_=bk[:].rearrange("(mo p) -> p mo", p=128))
    bv_row = sb.tile([1, CH], F32, name="bv_row")
    nc.sync.dma_start(out=bv_row[:], in_=bv[None, :])
    bv_bc = sb.tile([128, CH], F32, name="bv_bc")
    nc.gpsimd.partition_broadcast(bv_bc[:], bv_row[:])

    # mask as a per-row 0/1 multiplier on v_aug (kills masked keys in both
    # the attention numerator and the ones-column denominator)
    mask_sb = sb.tile([128, 2 * KC], I32, name="mask_sb")
    nc.sync.dma_start(
        out=mask_sb[:],
        in_=mask[:].rearrange("pp (kc p) -> p pp kc", p=128),
    )
    mbias = sb.tile([128, 2 * KC], F32, name="mbias")
    # (mask - 1) * MASK_NEG: 0 where mask==1, -MASK_NEG where mask==0
    nc.vector.tensor_scalar(
        out=mbias[:], in0=mask_sb[:], scalar1=-1, scalar2=MASK_NEG,
        op0=OP.add, op1=OP.mult,
    )

    ones_t = sb.tile([1, 64], F32, name="ones_t")
    nc.vector.memset(ones_t[:], 1.0)

    # ---- projections --------------------------------------------------
    # qT/kT: [o_local, p*S + s] in 2 tiles of 128 channels (2 heads each)
    qT = [sb.tile([128, R], QK_DT, name=f"qT{mo}") for mo in range(2)]
    kT = [sb.tile([128, R], QK_DT, name=f"kT{mo}") for mo in range(2)]
    # v_aug: [r_local, rc(16) x (h(4) x 65)]; col h*65+64 holds ones
    v_aug = sb.tile([128, 16 * NH * 65], AV_DT, name="v_aug")
    nc.gpsimd.memset(v_aug[:], 1.0)

    _stage_cache = {}

    def proj_chunk(kind, rb, part=None, lead=False):
        src_d = {"q": xq, "k": xk, "v": xv}[kind]
        w_sb = {"q": wq_sb, "k": wk_sb, "v": wv_sb}[kind]
        mos = (0, 1) if part is None else (part,)
        rss = (0, 1, 2, 3) if part is None else ((0, 1) if part == 0 else (2, 3))
        if (kind, rb) in _stage_cache:
            stage = _stage_cache[(kind, rb)]
        else:
            stage = sb.tile([128, KC * 512], X_DT, name="stage", tag="stage", bufs=3)
            _stage_cache[(kind, rb)] = stage
            engs = (nc.sync, nc.scalar) if lead else (nc.sync, nc.gpsimd)
            for half in range(2):
                eng = engs[half]
                eng.dma_start(
                    out=stage[:, half * 2048 : (half + 1) * 2048].rearrange(
                        "p (kc c) -> p kc c", kc=KC // 2
                    ),
                    in_=src_d[
                        half * 512 : 1024 if half else 512,
                        rb * 512 : (rb + 1) * 512,
                    ].rearrange("(kc p) c -> p kc c", p=128),
                )
        if kind in ("q", "k"):
            dst, b_sb = (qT, bq_sb) if kind == "q" else (kT, bk_sb)
            for mo in mos:
                pp_t = ps.tile([128, 512], F32, name="ps_proj", tag="ps_proj", bufs=2)
                for kc in range(KC):
                    nc.tensor.matmul(
                        pp_t[:],
                        w_sb[:, kc * CH + mo * 128 : kc * CH + (mo + 1) * 128],
                        stage[:, kc * 512 : (kc + 1) * 512],
                        start=(kc == 0),
                        stop=(kc == KC - 1),
                    )
                nc.vector.tensor_scalar(
                    out=dst[mo][:, rb * 512 : (rb + 1) * 512],
                    in0=pp_t[:],
                    scalar1=b_sb[:, mo : mo + 1],
                    scalar2=None,
                    op0=OP.add,
                )
        else:
            for rs in rss:
                rc = rb * 4 + rs
                pv_t = ps.tile([128, CH], F32, name="ps_v", tag="ps_proj", bufs=2)
                for kc in range(KC):
                    nc.tensor.matmul(
                        pv_t[:],
                        stage[:, kc * 512 + rs * 128 : kc * 512 + (rs + 1) * 128],
                        wv_sb[:, kc * CH : (kc + 1) * CH],
                        start=(kc == 0),
                        stop=(kc == KC - 1),
                    )
                dst_ap = v_aug[
                    :, rc * NH * 65 : (rc + 1) * NH * 65
                ].rearrange("p (h x) -> p h x", h=NH)[:, :, 0:DK]
                nc.vector.tensor_tensor(
                    out=dst_ap,
                    in0=pv_t[:].rearrange("p (h x) -> p h x", h=NH),
                    in1=bv_bc[:].rearrange("p (h x) -> p h x", h=NH),
                    op=OP.add,
                )

    # ---- attention building blocks ------------------------------------
    comb = [sb.tile([128, R], OUT_DT, name=f"comb{kk}") for kk in range(2)]

    def qk_part(p, h, side):
        pp = p if side == 0 else 1 - p
        mo, po = h // 2, (h % 2) * 64
        ex = [
            sb.tile([128, 4096], AV_DT, name="ex", tag="ex", bufs=3)
            for _ in range(2)
        ]
        for kc in range(KC):
            ss_t = ps.tile([128, 1024], F32, name="ps_s", tag="ps_s", bufs=3)
            for qb in range(2):
                nc.tensor.matmul(
                    ss_t[:, qb * 512 : (qb + 1) * 512],
                    kT[mo][po : po + 64, pp * S + kc * 128 : pp * S + (kc + 1) * 128],
                    qT[mo][po : po + 64, p * S + qb * 512 : p * S + (qb + 1) * 512],
                    start=True,
                    stop=True,
                )
            nc.scalar.activation(
                ex[kc // 4][:, (kc % 4) * 1024 : (kc % 4 + 1) * 1024],
                ss_t[:],
                AF.Exp,
                bias=mbias[:, pp * KC + kc : pp * KC + kc + 1],
                scale=0.125,
            )
        return ex

    def av_part(p, h, side, ex):
        pp = p if side == 0 else 1 - p
        av = sb.tile([65, S], F32, name="av", tag="avT", bufs=6)
        for qb in range(2):
            pa_t = ps.tile([65, 512], F32, name="ps_av", tag="ps_proj", bufs=2)
            for kc in range(KC):
                nc.tensor.matmul(
                    pa_t[:],
                    v_aug[:, (pp * KC + kc) * NH * 65 + h * 65 : (pp * KC + kc) * NH * 65 + (h + 1) * 65],
                    ex[kc // 4][:, (kc % 4) * 1024 + qb * 512 : (kc % 4) * 1024 + (qb + 1) * 512],
                    start=(kc == 0),
                    stop=(kc == KC - 1),
                )
            nc.vector.tensor_copy(av[:, qb * 512 : (qb + 1) * 512], pa_t[:])
        return av

    def combo(p, h, side):
        return av_part(p, h, side, qk_part(p, h, side))

    _pair = {}

    def combine(p, h, avL, avR, pe_bc=False):
        srs = sb.tile([32, 64], F32, name="srs", tag="srs", bufs=2)
        nc.sync.dma_start(
            out=srs[0:16, :], in_=avL[64:65, :].rearrange("p (m e) -> p m e", e=64)
        )
        nc.sync.dma_start(
            out=srs[16:32, :], in_=avR[64:65, :].rearrange("p (m e) -> p m e", e=64)
        )
        rrs = sb.tile([32, 64], F32, name="rrs", tag="rrs", bufs=2)
        nc.vector.reciprocal(rrs[:], srs[:])
        rr2 = sb.tile([1, 2 * S], F32, name="rr2", tag="rrow", bufs=2)
        nc.sync.dma_start(
            out=rr2[:, 0:S].rearrange("p (m e) -> p m e", e=64), in_=rrs[0:16, :]
        )
        nc.sync.dma_start(
            out=rr2[:, S : 2 * S].rearrange("p (m e) -> p m e", e=64), in_=rrs[16:32, :]
        )
        def part2():
            po = (h % 2) * 64
            if pe_bc:
                bcA = ps.tile([64, S], F32, name="bcA", tag="ps_s", bufs=3)
                bcB = ps.tile([64, S], F32, name="bcB", tag="ps_s", bufs=3)
                for c in range(4):
                    dst = bcA if c < 2 else bcB
                    nc.tensor.matmul(
                        dst[:, (c % 2) * 512 : (c % 2 + 1) * 512],
                        ones_t[:].bitcast(F32R),
                        rr2[0:1, c * 512 : (c + 1) * 512].bitcast(F32R),
                        start=True,
                        stop=True,
                    )
                bcL_ap, bcR_ap = bcA[:], bcB[:]
            else:
                bc2 = sb.tile([64, 2 * S], F32, name="bc2", tag="bc", bufs=2)
                nc.gpsimd.partition_broadcast(bc2[:], rr2[:])
                bcL_ap, bcR_ap = bc2[:, 0:S], bc2[:, S : 2 * S]
            if h % 2 == 0:
                t1p = sb.tile([128, S], F32, name="t1p", tag="t1", bufs=2)
                t2p = sb.tile([128, S], F32, name="t2p", tag="t2", bufs=2)
                _pair[(p, h // 2)] = (t1p, t2p)
            else:
                t1p, t2p = _pair[(p, h // 2)]
            nc.vector.tensor_tensor(
                out=t1p[po : po + 64, :], in0=avL[0:64, :], in1=bcL_ap, op=OP.mult
            )
            nc.vector.tensor_tensor(
                out=t2p[po : po + 64, :], in0=avR[0:64, :], in1=bcR_ap, op=OP.mult
            )
            if h % 2 == 1:
                t3p = sb.tile([128, S], F32, name="t3p", tag="t3", bufs=2)
                nc.scalar.activation(t3p[:], t2p[:], AF.Tanh)
                nc.vector.scalar_tensor_tensor(
                    out=comb[h // 2][:, p * S : (p + 1) * S],
                    in0=t3p[:],
                    scalar=0.1,
                    in1=t1p[:],
                    op0=OP.mult,
                    op1=OP.add,
                )

        return part2

    def outproj_rc(p, rc):
        od = sb.tile([128, D], F32, name="od", tag="od", bufs=2)
        for ob in range(2):
            po_t = ps.tile([128, 512], F32, name="ps_o", tag="ps_proj", bufs=2)
            for kk in range(2):
                nc.tensor.matmul(
                    po_t[:],
                    comb[kk][:, p * S + rc * 128 : p * S + (rc + 1) * 128],
                    wo_sb[:, kk * D + ob * 512 : kk * D + (ob + 1) * 512],
                    start=(kk == 0),
                    stop=(kk == 1),
                )
            nc.vector.tensor_copy(od[:, ob * 512 : (ob + 1) * 512], po_t[:])
        (nc.sync if rc % 2 == 0 else nc.gpsimd).dma_start(
            out=out_d[p * S + rc * 128 : p * S + (rc + 1) * 128, :], in_=od[:]
        )

    # ---- schedule -----------------------------------------------------
    # lead-in: k and q p0-halves; the v projection hides inside the first
    # combo (emitted between its QK/exp chain and its AV stage)
    for kind, rb in (("k", 0), ("k", 1), ("q", 0), ("q", 1)):
        proj_chunk(kind, rb, lead=True)
    ex00 = qk_part(0, 0, 0)
    proj_chunk("v", 0, lead=True)
    proj_chunk("v", 1, lead=True)
    av0 = {0: av_part(0, 0, 0, ex00)}

    # one projection chunk interleaved per combo; all of k/v's p1 halves
    # must land before the first side-1 combo reads them
    rest = [("k", 2), ("k", 3), ("v", 2)]
    for h in range(1, NH):
        av0[h] = combo(0, h, 0)
        proj_chunk(*rest.pop(0))
    proj_chunk("v", 3)
    rest = [("q", 2), ("q", 3)]
    pending = None
    for h in range(NH):
        avR = combo(0, h, 1)
        if pending:
            pending()
        if rest:
            proj_chunk(*rest.pop(0))
        pending = combine(0, h, av0[h], avR)

    # p1 attention: p0's output projection fills the ACT-bound stream;
    # combine part-2 chains are deferred one combo so QK work covers them
    for h in range(NH):
        avL = combo(1, h, 0)
        if pending:
            pending()
        if h < 3:
            outproj_rc(0, 2 * h)
        avR = combo(1, h, 1)
        if h < 3:
            outproj_rc(0, 2 * h + 1)
        pending = combine(1, h, avL, avR, pe_bc=(h == 3))
    for rc in (6, 7):
        outproj_rc(0, rc)
    pending()
    for rc in range(8):
        outproj_rc(1, rc)


_CACHED = None


def _build():
    global _CACHED
    if _CACHED is not None:
        return _CACHED
    nc = bacc.Bacc("TRN2", target_bir_lowering=False, debug=False)
    xq = nc.dram_tensor("xq", [D, R], X_DT, kind="ExternalInput")
    xk = nc.dram_tensor("xk", [D, R], X_DT, kind="ExternalInput")
    xv = nc.dram_tensor("xv", [D, R], X_DT, kind="ExternalInput")
    wq = nc.dram_tensor("wq", [D, CH], X_DT, kind="ExternalInput")
    wk = nc.dram_tensor("wk", [D, CH], X_DT, kind="ExternalInput")
    wv = nc.dram_tensor("wv", [D, CH], X_DT, kind="ExternalInput")
    wo = nc.dram_tensor("wo", [CH, D], OUT_DT, kind="ExternalInput")
    bq = nc.dram_tensor("bq", [CH], F32, kind="ExternalInput")
    bk = nc.dram_tensor("bk", [CH], F32, kind="ExternalInput")
    bv = nc.dram_tensor("bv", [CH], F32, kind="ExternalInput")
    mask = nc.dram_tensor("mask", [P, S], I32, kind="ExternalInput")
    out_d = nc.dram_tensor("out", [R, D], F32, kind="ExternalOutput")
    with tile.TileContext(nc) as tc:
        _emit(nc, tc, xq, xk, xv, wq, wk, wv, wo, bq, bk, bv, mask, out_d)
    nc.compile()
    _CACHED = nc
    return nc


def _in_maps(query, key, value, mask, Wq, bq, Wk, bk, Wv, bv, Wo):
    xnp = mybir.dt.np(X_DT)
    onp = mybir.dt.np(OUT_DT)
    f32 = lambda a: np.ascontiguousarray(np.asarray(a, dtype=np.float32))
    xdt = lambda a: np.ascontiguousarray(np.asarray(a).astype(xnp))
    odt = lambda a: np.ascontiguousarray(np.asarray(a).astype(onp))
    query, key, value = f32(query), f32(key), f32(value)
    Wq, Wk, Wv, Wo = f32(Wq), f32(Wk), f32(Wv), f32(Wo)
    bq, bk, bv = f32(bq), f32(bk), f32(bv)
    mask = np.ascontiguousarray(np.asarray(mask, dtype=np.int32))

    xqT = [xdt(query[b].reshape(R, D).T) for b in range(B)]
    xkT = [xdt(key[b].reshape(R, D).T) for b in range(B)]
    xvT = [xdt(value[b].reshape(R, D).T) for b in range(B)]

    maps = []
    for c in range(NCORES):
        b, hg = divmod(c, GROUP)
        ch = slice(hg * CH, (hg + 1) * CH)
        maps.append(
            {
                "xq": xqT[b],
                "xk": xkT[b],
                "xv": xvT[b],
                "wq": xdt(Wq[ch, :].T),
                "wk": xdt(Wk[ch, :].T),
                "wv": xdt(Wv[ch, :].T),
                "wo": odt(Wo[:, ch].T),
                "bq": bq[ch],
                "bk": bk[ch],
                "bv": bv[ch],
                "mask": mask[b, :, 0, :],
            }
        )
    return maps


def _run(in_maps, **kwargs):
    nc = _build()
    return run_bass_kernel_spmd(nc, in_maps, core_ids=list(range(NCORES)), **kwargs)


def kernel(query, key, value, mask, Wq, bq, Wk, bk, Wv, bv, Wo, bo):
    res = _run(_in_maps(query, key, value, mask, Wq, bq, Wk, bk, Wv, bv, Wo))
    bo = np.asarray(bo, dtype=np.float32)
    out = np.zeros((B, P, S, D), dtype=np.float32)
    for c in range(NCORES):
        b = c // GROUP
        out[b] += res.results[c]["out"].reshape(P, S, D)
    out += bo
    return out


# revision 32
# speedup vs baseline: 1.0690x; 1.0074x over previous
"""Trainium2 Bass kernel for the paired-view ("flip") multi-head attention module.

Full computation (reference semantics, B=2 P=2 S=1024 D=1024 H=16):
    q/k/v = Linear(x) -> [B,P,H,S,DK]
    left  = softmax(q k^T / 8 + mask) v          (same pair index)
    right = softmax(q k_flip^T / 8 + mask) v_flip (pair index swapped)
    out   = (left + 0.1*tanh(right)) @ Wo.T + bo

Sharding over 8 NeuronCores: data-parallel on B (2 groups of 4 cores),
tensor-parallel on heads within a group (4 heads/core, 256 channels).
Each core computes its heads' projections (column-parallel), full attention
for its heads over both pair views, and a row-parallel partial of the output
projection.  The host sums the 4 partials per batch and adds bo.

Key layout trick: scores are computed TRANSPOSED ([k, q] instead of [q, k])
so softmax's exp is orientation-free and the attention-value product needs
no on-chip transposes; row sums come free via an extra ones-column in V.
Matmuls run in bf16 (fp32 PSUM accumulate); attention is ScalarE-exp-bound,
so projections and the p0 output projection are interleaved into the
ACT-saturated stream, and softmax reciprocals are batched by folding the
sums rows onto 32 partitions.
"""

import numpy as np

import concourse.bass as bass
import concourse.tile as tile
from concourse import bacc, mybir
from concourse.bass_utils import run_bass_kernel_spmd

F32 = mybir.dt.float32
F32R = mybir.dt.float32r
BF16 = mybir.dt.bfloat16
I32 = mybir.dt.int32

# per-stage matmul input dtypes (both operands of a matmul must match)
X_DT = BF16    # projection inputs: xT staging + Wq/Wk/Wv
QK_DT = BF16   # q/k tiles feeding the scores matmul
AV_DT = BF16   # exp(scores) + v_aug feeding the AV matmul
OUT_DT = BF16  # combine + Wo feeding the output projection
AF = mybir.ActivationFunctionType
OP = mybir.AluOpType

B, P, S, D, H = 2, 2, 1024, 1024, 16
DK = D // H          # 64
NCORES = 8
GROUP = 4            # cores per batch entry
NH = H // GROUP      # 4 local heads per core
CH = NH * DK         # 256 local channels
R = P * S            # 2048 rows per batch entry
KC = 8               # d_model chunks of 128
RB = 4               # row banks of 512
MASK_NEG = 60.0      # exp(-60) == 0 relative to any sum


def _emit(nc, tc, xq, xk, xv, wq, wk, wv, wo, bq, bk, bv, mask, out_d):
    from contextlib import ExitStack

    with ExitStack() as ctx:
        sb = ctx.enter_context(tc.tile_pool(name="sb", bufs=1))
        ps = ctx.enter_context(tc.tile_pool(name="ps", bufs=1, space="PSUM"))
        _body(nc, sb, ps, xq, xk, xv, wq, wk, wv, wo, bq, bk, bv, mask, out_d)


def _body(nc, sb, ps, xq, xk, xv, wq, wk, wv, wo, bq, bk, bv, mask, out_d):
    # ---- constants ----------------------------------------------------
    wq_sb = sb.tile([128, KC * CH], X_DT, name="wq_sb")
    wk_sb = sb.tile([128, KC * CH], X_DT, name="wk_sb")
    wv_sb = sb.tile([128, KC * CH], X_DT, name="wv_sb")
    for t_d, t_s in ((wq, wq_sb), (wk, wk_sb), (wv, wv_sb)):
        nc.gpsimd.dma_start(
            out=t_s[:].rearrange("p (kc c) -> p kc c", kc=KC),
            in_=t_d[:].rearrange("(kc p) c -> p kc c", p=128),
        )
    wo_sb = sb.tile([128, 2 * D], OUT_DT, name="wo_sb")
    nc.gpsimd.dma_start(
        out=wo_sb[:].rearrange("p (kk c) -> p kk c", kk=2),
        in_=wo[:].rearrange("(kk p) c -> p kk c", p=128),
    )

    bq_sb = sb.tile([128, 2], F32, name="bq_sb")
    bk_sb = sb.tile([128, 2], F32, name="bk_sb")
    nc.sync.dma_start(out=bq_sb[:], in_=bq[:].rearrange("(mo p) -> p mo", p=128))
    nc.sync.dma_start(out=bk_sb[:], in_=bk[:].rearrange("(mo p) -> p mo", p=128))
    bv_row = sb.tile([1, CH], F32, name="bv_row")
    nc.sync.dma_start(out=bv_row[:], in_=bv[None, :])
    bv_bc = sb.tile([128, CH], F32, name="bv_bc")
    nc.gpsimd.partition_broadcast(bv_bc[:], bv_row[:])

    # mask as a per-row 0/1 multiplier on v_aug (kills masked keys in both
    # the attention numerator and the ones-column denominator)
    mask_sb = sb.tile([128, 2 * KC], I32, name="mask_sb")
    nc.sync.dma_start(
        out=mask_sb[:],
        in_=mask[:].rearrange("pp (kc p) -> p pp kc", p=128),
    )
    mbias = sb.tile([128, 2 * KC], F32, name="mbias")
    # (mask - 1) * MASK_NEG: 0 where mask==1, -MASK_NEG where mask==0
    nc.vector.tensor_scalar(
        out=mbias[:], in0=mask_sb[:], scalar1=-1, scalar2=MASK_NEG,
        op0=OP.add, op1=OP.mult,
    )

    ones_t = sb.tile([1, 64], F32, name="ones_t")
    nc.vector.memset(ones_t[:], 1.0)

    # ---- projections --------------------------------------------------
    # qT/kT: [o_local, p*S + s] in 2 tiles of 128 channels (2 heads each)
    qT = [sb.tile([128, R], QK_DT, name=f"qT{mo}") for mo in range(2)]
    kT = [sb.tile([128, R], QK_DT, name=f"kT{mo}") for mo in range(2)]
    # v_aug: [r_local, rc(16) x (h(4) x 65)]; col h*65+64 holds ones
    v_aug = sb.tile([128, 16 * NH * 65], AV_DT, name="v_aug")
    nc.gpsimd.memset(v_aug[:], 1.0)

    _stage_cache = {}

    def proj_chunk(kind, rb, part=None, lead=False):
        src_d = {"q": xq, "k": xk, "v": xv}[kind]
        w_sb = {"q": wq_sb, "k": wk_sb, "v": wv_sb}[kind]
        mos = (0, 1) if part is None else (part,)
        rss = (0, 1, 2, 3) if part is None else ((0, 1) if part == 0 else (2, 3))
        if (kind, rb) in _stage_cache:
            stage = _stage_cache[(kind, rb)]
        else:
            stage = sb.tile([128, KC * 512], X_DT, name="stage", tag="stage", bufs=3)
            _stage_cache[(kind, rb)] = stage
            engs = (nc.sync, nc.scalar) if lead else (nc.sync, nc.gpsimd)
            for half in range(2):
                eng = engs[half]
                eng.dma_start(
                    out=stage[:, half * 2048 : (half + 1) * 2048].rearrange(
                        "p (kc c) -> p kc c", kc=KC // 2
                    ),
                    in_=src_d[
                        half * 512 : 1024 if half else 512,
                        rb * 512 : (rb + 1) * 512,
                    ].rearrange("(kc p) c -> p kc c", p=128),
                )
        if kind in ("q", "k"):
            dst, b_sb = (qT, bq_sb) if kind == "q" else (kT, bk_sb)
            for mo in mos:
                pp_t = ps.tile([128, 512], F32, name="ps_proj", tag="ps_proj", bufs=2)
                for kc in range(KC):
                    nc.tensor.matmul(
                        pp_t[:],
                        w_sb[:, kc * CH + mo * 128 : kc * CH + (mo + 1) * 128],
                        stage[:, kc * 512 : (kc + 1) * 512],
                        start=(kc == 0),
                        stop=(kc == KC - 1),
                    )
                nc.vector.tensor_scalar(
                    out=dst[mo][:, rb * 512 : (rb + 1) * 512],
                    in0=pp_t[:],
                    scalar1=b_sb[:, mo : mo + 1],
                    scalar2=None,
                    op0=OP.add,
                )
        else:
            for rs in rss:
                rc = rb * 4 + rs
                pv_t = ps.tile([128, CH], F32, name="ps_v", tag="ps_proj", bufs=2)
                for kc in range(KC):
                    nc.tensor.matmul(
                        pv_t[:],
                        stage[:, kc * 512 + rs * 128 : kc * 512 + (rs + 1) * 128],
                        wv_sb[:, kc * CH : (kc + 1) * CH],
                        start=(kc == 0),
                        stop=(kc == KC - 1),
                    )
                dst_ap = v_aug[
                    :, rc * NH * 65 : (rc + 1) * NH * 65
                ].rearrange("p (h x) -> p h x", h=NH)[:, :, 0:DK]
                nc.vector.tensor_tensor(
                    out=dst_ap,
                    in0=pv_t[:].rearrange("p (h x) -> p h x", h=NH),
                    in1=bv_bc[:].rearrange("p (h x) -> p h x", h=NH),
                    op=OP.add,
                )

    # ---- attention building blocks ------------------------------------
    comb = [sb.tile([128, R], OUT_DT, name=f"comb{kk}") for kk in range(2)]

    def qk_part(p, h, side):
        pp = p if side == 0 else 1 - p
        mo, po = h // 2, (h % 2) * 64
        ex = [
            sb.tile([128, 4096], AV_DT, name="ex", tag="ex", bufs=4)
            for _ in range(2)
        ]
        for kc in range(KC):
            ss_t = ps.tile([128, 1024], F32, name="ps_s", tag="ps_s", bufs=3)
            for qb in range(2):
                nc.tensor.matmul(
                    ss_t[:, qb * 512 : (qb + 1) * 512],
                    kT[mo][po : po + 64, pp * S + kc * 128 : pp * S + (kc + 1) * 128],
                    qT[mo][po : po + 64, p * S + qb * 512 : p * S + (qb + 1) * 512],
                    start=True,
                    stop=True,
                )
            nc.scalar.activation(
                ex[kc // 4][:, (kc % 4) * 1024 : (kc % 4 + 1) * 1024],
                ss_t[:],
                AF.Exp,
                bias=mbias[:, pp * KC + kc : pp * KC + kc + 1],
                scale=0.125,
            )
        return ex

    def av_part(p, h, side, ex):
        pp = p if side == 0 else 1 - p
        av = sb.tile([65, S], F32, name="av", tag="avT", bufs=6)
        for qb in range(2):
            pa_t = ps.tile([65, 512], F32, name="ps_av", tag="ps_proj", bufs=2)
            for kc in range(KC):
                nc.tensor.matmul(
                    pa_t[:],
                    v_aug[:, (pp * KC + kc) * NH * 65 + h * 65 : (pp * KC + kc) * NH * 65 + (h + 1) * 65],
                    ex[kc // 4][:, (kc % 4) * 1024 + qb * 512 : (kc % 4) * 1024 + (qb + 1) * 512],
                    start=(kc == 0),
                    stop=(kc == KC - 1),
                )
            nc.vector.tensor_copy(av[:, qb * 512 : (qb + 1) * 512], pa_t[:])
        return av

    def combo(p, h, side):
        return av_part(p, h, side, qk_part(p, h, side))

    _pair = {}

    def combine(p, h, avL, avR, pe_bc=False):
        srs = sb.tile([32, 64], F32, name="srs", tag="srs", bufs=2)
        nc.sync.dma_start(
            out=srs[0:16, :], in_=avL[64:65, :].rearrange("p (m e) -> p m e", e=64)
        )
        nc.sync.dma_start(
            out=srs[16:32, :], in_=avR[64:65, :].rearrange("p (m e) -> p m e", e=64)
        )
        rrs = sb.tile([32, 64], F32, name="rrs", tag="rrs", bufs=2)
        nc.vector.reciprocal(rrs[:], srs[:])
        rr2 = sb.tile([1, 2 * S], F32, name="rr2", tag="rrow", bufs=2)
        nc.sync.dma_start(
            out=rr2[:, 0:S].rearrange("p (m e) -> p m e", e=64), in_=rrs[0:16, :]
        )
        nc.sync.dma_start(
            out=rr2[:, S : 2 * S].rearrange("p (m e) -> p m e", e=64), in_=rrs[16:32, :]
        )
        def part2():
            po = (h % 2) * 64
            if pe_bc:
                bcA = ps.tile([64, S], F32, name="bcA", tag="ps_s", bufs=3)
                bcB = ps.tile([64, S], F32, name="bcB", tag="ps_s", bufs=3)
                for c in range(4):
                    dst = bcA if c < 2 else bcB
                    nc.tensor.matmul(
                        dst[:, (c % 2) * 512 : (c % 2 + 1) * 512],
                        ones_t[:].bitcast(F32R),
                        rr2[0:1, c * 512 : (c + 1) * 512].bitcast(F32R),
                        start=True,
                        stop=True,
                    )
                bcL_ap, bcR_ap = bcA[:], bcB[:]
            else:
                bc2 = sb.tile([64, 2 * S], F32, name="bc2", tag="bc", bufs=2)
                nc.gpsimd.partition_broadcast(bc2[:], rr2[:])
                bcL_ap, bcR_ap = bc2[:, 0:S], bc2[:, S : 2 * S]
            if h % 2 == 0:
                t1p = sb.tile([128, S], F32, name="t1p", tag="t1", bufs=2)
                t2p = sb.tile([128, S], F32, name="t2p", tag="t2", bufs=2)
                _pair[(p, h // 2)] = (t1p, t2p)
            else:
                t1p, t2p = _pair[(p, h // 2)]
            nc.vector.tensor_tensor(
                out=t1p[po : po + 64, :], in0=avL[0:64, :], in1=bcL_ap, op=OP.mult
            )
            nc.vector.tensor_tensor(
                out=t2p[po : po + 64, :], in0=avR[0:64, :], in1=bcR_ap, op=OP.mult
            )
            if h % 2 == 1:
                t3p = sb.tile([128, S], F32, name="t3p", tag="t3", bufs=2)
                nc.scalar.activation(t3p[:], t2p[:], AF.Tanh)
                nc.vector.scalar_tensor_tensor(
                    out=comb[h // 2][:, p * S : (p + 1) * S],
                    in0=t3p[:],
                    scalar=0.1,
                    in1=t1p[:],
                    op0=OP.mult,
                    op1=OP.add,
                )

        return part2

    def outproj_rc(p, rc, act_copy=False):
        od = sb.tile([128, D], F32, name="od", tag="od", bufs=2)
        for ob in range(2):
            po_t = ps.tile([128, 512], F32, name="ps_o", tag="ps_proj", bufs=2)
            for kk in range(2):
                nc.tensor.matmul(
                    po_t[:],
                    comb[kk][:, p * S + rc * 128 : p * S + (rc + 1) * 128],
                    wo_sb[:, kk * D + ob * 512 : kk * D + (ob + 1) * 512],
                    start=(kk == 0),
                    stop=(kk == 1),
                )
            if act_copy and ob == 1:
                nc.scalar.copy(od[:, ob * 512 : (ob + 1) * 512], po_t[:])
            else:
                nc.vector.tensor_copy(od[:, ob * 512 : (ob + 1) * 512], po_t[:])
        (nc.sync if rc % 2 == 0 else nc.gpsimd).dma_start(
            out=out_d[p * S + rc * 128 : p * S + (rc + 1) * 128, :], in_=od[:]
        )

    # ---- schedule -----------------------------------------------------
    # lead-in: k and q p0-halves; the v projection hides inside the first
    # combo (emitted between its QK/exp chain and its AV stage)
    for kind, rb in (("k", 0), ("k", 1), ("q", 0), ("q", 1)):
        proj_chunk(kind, rb, lead=True)
    ex00 = qk_part(0, 0, 0)
    proj_chunk("v", 0, lead=True)
    proj_chunk("v", 1, lead=True)
    av0 = {0: av_part(0, 0, 0, ex00)}

    # one projection chunk interleaved per combo; all of k/v's p1 halves
    # must land before the first side-1 combo reads them
    rest = [("k", 2), ("k", 3), ("v", 2)]
    for h in range(1, NH):
        av0[h] = combo(0, h, 0)
        proj_chunk(*rest.pop(0))
    proj_chunk("v", 3)
    rest = [("q", 2), ("q", 3)]
    pending = None
    for h in range(NH):
        avR = combo(0, h, 1)
        if pending:
            pending()
        if rest:
            proj_chunk(*rest.pop(0))
        pending = combine(0, h, av0[h], avR)

    # p1 attention: p0's output projection fills the ACT-bound stream;
    # combine part-2 chains are deferred one combo so QK work covers them
    for h in range(NH):
        avL = combo(1, h, 0)
        if pending:
            pending()
        if h < 3:
            outproj_rc(0, 2 * h)
        avR = combo(1, h, 1)
        if h < 3:
            outproj_rc(0, 2 * h + 1)
        pending = combine(1, h, avL, avR, pe_bc=(h == 3))
    for rc in (6, 7):
        outproj_rc(0, rc)
    pending()
    for rc in range(8):
        outproj_rc(1, rc, act_copy=True)


_CACHED = None


def _build():
    global _CACHED
    if _CACHED is not None:
        return _CACHED
    nc = bacc.Bacc("TRN2", target_bir_lowering=False, debug=False)
    xq = nc.dram_tensor("xq", [D, R], X_DT, kind="ExternalInput")
    xk = nc.dram_tensor("xk", [D, R], X_DT, kind="ExternalInput")
    xv = nc.dram_tensor("xv", [D, R], X_DT, kind="ExternalInput")
    wq = nc.dram_tensor("wq", [D, CH], X_DT, kind="ExternalInput")
    wk = nc.dram_tensor("wk", [D, CH], X_DT, kind="ExternalInput")
    wv = nc.dram_tensor("wv", [D, CH], X_DT, kind="ExternalInput")
    wo = nc.dram_tensor("wo", [CH, D], OUT_DT, kind="ExternalInput")
    bq = nc.dram_tensor("bq", [CH], F32, kind="ExternalInput")
    bk = nc.dram_tensor("bk", [CH], F32, kind="ExternalInput")
    bv = nc.dram_tensor("bv", [CH], F32, kind="ExternalInput")
    mask = nc.dram_tensor("mask", [P, S], I32, kind="ExternalInput")
    out_d = nc.dram_tensor("out", [R, D], F32, kind="ExternalOutput")
    with tile.TileContext(nc) as tc:
        _emit(nc, tc, xq, xk, xv, wq, wk, wv, wo, bq, bk, bv, mask, out_d)
    nc.compile()
    _CACHED = nc
    return nc


def _in_maps(query, key, value, mask, Wq, bq, Wk, bk, Wv, bv, Wo):
    xnp = mybir.dt.np(X_DT)
    onp = mybir.dt.np(OUT_DT)
    f32 = lambda a: np.ascontiguousarray(np.asarray(a, dtype=np.float32))
    xdt = lambda a: np.ascontiguousarray(np.asarray(a).astype(xnp))
    odt = lambda a: np.ascontiguousarray(np.asarray(a).astype(onp))
    query, key, value = f32(query), f32(key), f32(value)
    Wq, Wk, Wv, Wo = f32(Wq), f32(Wk), f32(Wv), f32(Wo)
    bq, bk, bv = f32(bq), f32(bk), f32(bv)
    mask = np.ascontiguousarray(np.asarray(mask, dtype=np.int32))

    xqT = [xdt(query[b].reshape(R, D).T) for b in range(B)]
    xkT = [xdt(key[b].reshape(R, D).T) for b in range(B)]
    xvT = [xdt(value[b].reshape(R, D).T) for b in range(B)]

    maps = []
    for c in range(NCORES):
        b, hg = divmod(c, GROUP)
        ch = slice(hg * CH, (hg + 1) * CH)
        maps.append(
            {
                "xq": xqT[b],
                "xk": xkT[b],
                "xv": xvT[b],
                "wq": xdt(Wq[ch, :].T),
                "wk": xdt(Wk[ch, :].T),
                "wv": xdt(Wv[ch, :].T),
                "wo": odt(Wo[:, ch].T),
                "bq": bq[ch],
                "bk": bk[ch],
                "bv": bv[ch],
                "mask": mask[b, :, 0, :],
            }
        )
    return maps


def _run(in_maps, **kwargs):
    nc = _build()
    return run_bass_kernel_spmd(nc, in_maps, core_ids=list(range(NCORES)), **kwargs)


def kernel(query, key, value, mask, Wq, bq, Wk, bk, Wv, bv, Wo, bo):
    res = _run(_in_maps(query, key, value, mask, Wq, bq, Wk, bk, Wv, bv, Wo))
    bo = np.asarray(bo, dtype=np.float32)
    out = np.zeros((B, P, S, D), dtype=np.float32)
    for c in range(NCORES):
        b = c // GROUP
        out[b] += res.results[c]["out"].reshape(P, S, D)
    out += bo
    return out
